# revision 8
# baseline (speedup 1.0000x reference)
"""Trainium2 Bass kernel for nn_BktModel (soft-membership BKT HMM forward).

Exact rank-64 factorization of the BKT recurrence + single warm-started
Jacobi sweep over 25-step sub-blocks, software-pipelined one sub-block
ahead, with work spread across VectorE, GPSIMD and ScalarE.

Math, per batch row (window of G=125 steps, cp_t = prod_{i<t}(1-cc_i)):
  Q_t = cc_t*cp_t, V_t = cc_t/(cp_t*(1-cc_t)); the log-alpha state folds
  into h[s] = la0[s] + sum_j V_j a3_j[s] with probes a2_t[s] = <Q_t, h[s]>;
  in-sub-block coupling R = Q V^T (strictly lower triangular) is resolved
  by ONE Jacobi sweep warm-started from the previous sub-block:
    a2 = probe(h_{m-2}) + (R_cross + R_tri) @ a3_{m-1}
  (R_cross restores the pipelined probe's missing history exactly; R_tri
  is the warm start; the probe runs against a two-sub-block-old h so it
  overlaps the previous sub-block's sweep). Numpy mirror 7.5e-3;
  hardware 1.17e-2 vs budget 2e-2. Window boundaries rescale h by
  P = cp_G (host pre-scales Q and R_cross for window-opening sub-blocks).

Engine placement (all op classes hardware-verified): rank-64
contractions (probe Q*h, update V*a3) and the HxH matvec split by HMM
state - state 0 on VectorE, state 1 on GPSIMD (bf16 broadcast-multiply
+ fold-tree adds); final reduces on VectorE; exp/ln plus the h->bf16
copy and a3 pad memzero on ScalarE; output-path sums and the h update
on GPSIMD. gpsimd memset-on-slice / dtype-converting copies crash NRT
and are avoided.

Sharding: data-parallel over batch; 8 cores x 128 rows (partitions).

Measured: CoreSim cost model 154.7us/core (first working baseline:
843us, 5.45x); hardware rel err 1.17e-02.
"""

import os
import sys
import threading

import numpy as np

for _p in ("/opt/trn_rl_repo", "/root/.axon_site/_ro/trn_rl_repo"):
    if os.path.isdir(_p) and _p not in sys.path:
        sys.path.append(_p)

B, T, C, K = 1024, 500, 64, 2000
S, O = 2, 2
N_CORES = 8
BL = B // N_CORES
H = 25
JP = 28
G = 125
NSB = T // H
NW = T // G
NSBW = NSB // NW               # sub-blocks per window
SBW = H * C + C * JP + H * JP   # Q | V | Rsum

_cache = {}
_lock = threading.Lock()


def _build_program():
    import concourse.bass as bass
    import concourse.mybir as mybir
    import concourse.tile as tile
    from concourse import bacc

    Act = mybir.ActivationFunctionType
    _orig_tables = bacc.get_activation_tables

    def _tables_combined_exp_ln(arch):
        tabs = _orig_tables(arch)
        out = {}
        for name, fns in tabs.items():
            if name == "natural_log_exp_and_others":
                out[name] = fns
            else:
                out[name] = {f for f in fns if f not in (Act.Exp, Act.Ln)}
        return out

    bacc.get_activation_tables = _tables_combined_exp_ln
    try:
        return _build_program_inner(bass, mybir, tile, bacc)
    finally:
        bacc.get_activation_tables = _orig_tables


def _build_program_inner(bass, mybir, tile, bacc):
    f32 = mybir.dt.float32
    bf16 = mybir.dt.bfloat16
    Alu = mybir.AluOpType
    Act = mybir.ActivationFunctionType
    Ax = mybir.AxisListType

    nc = bacc.Bacc("TRN2", target_bir_lowering=False, debug=False)
    with tile.TileContext(nc) as tc:
        with tc.tile_pool(name="dram", bufs=1, space="DRAM") as dram:
            qvr = dram.tile([BL, NSB, SBW], bf16, kind="ExternalInput", name="qvr")
            eaxs = dram.tile([BL, NSB, 2 * H * 4], f32, kind="ExternalInput", name="eaxs")
            lainit = dram.tile([BL, 2 * C], f32, kind="ExternalInput", name="lainit")
            pws = dram.tile([BL, NW, C], f32, kind="ExternalInput", name="pws")
            out = dram.tile([BL, 2 * T], f32, kind="ExternalOutput", name="out")

            with (
                tc.tile_pool(name="persist", bufs=1) as pp,
                tc.tile_pool(name="stream", bufs=3) as gp,
                tc.tile_pool(name="small", bufs=3) as sp,
                tc.tile_pool(name="big", bufs=2) as bp,
            ):
                hb0 = pp.tile([BL, 2 * C], f32, name="h0")
                hb1 = pp.tile([BL, 2 * C], f32, name="h1")
                hbufs = [hb0, hb1]
                nc.sync.dma_start(hb0[:], lainit[:])
                nc.sync.dma_start(hb1[:], lainit[:])
                pwt = pp.tile([BL, NW, C], f32, name="pwt")
                nc.sync.dma_start(pwt[:], pws[:])
                a3z = pp.tile([BL, 2 * JP], bf16, name="a3z")
                nc.vector.memset(a3z[:], 0.0)
                a3bf_prev = a3z

                for sb in range(NSB):
                    w, m = divmod(sb, NSBW)
                    qv = gp.tile([BL, SBW], bf16, name="qv", tag="qv")
                    nc.sync.dma_start(qv[:], qvr[:, sb, :])
                    ea = gp.tile([BL, 2 * H * 4], f32, name="ea", tag="ea")
                    nc.sync.dma_start(ea[:], eaxs[:, sb, :])
                    qs = qv[:, 0 : H * C].rearrange("p (a k c) -> p a k c", a=1, c=C)
                    o1 = H * C
                    vs0 = qv[:, o1 : o1 + C * JP].rearrange("p (c j) -> p c j", j=JP)
                    o2 = o1 + C * JP
                    rsum = qv[:, o2 : o2 + H * JP].rearrange(
                        "p (a k j) -> p a k j", a=1, j=JP
                    )
                    ea4 = ea[:].rearrange("p (s k i) -> p s k i", s=2, i=4)

                    # --- early probe vs h_{sb-2} (state-split) ---
                    hsrc = hbufs[sb % 2]
                    hbt = sp.tile([BL, 2 * C], bf16, name="hbt", tag="hbt")
                    nc.scalar.copy(hbt[:], hsrc[:])
                    hbv2 = hbt[:].rearrange("p (s c) -> p s c", s=2)
                    qs3 = qv[:, 0 : H * C].rearrange("p (k c) -> p k c", c=C)
                    bq = sp.tile([BL, 2 * H], f32, name="bq", tag="bq")
                    bqv = bq[:].rearrange("p (s k) -> p s k", s=2)
                    for s_, eng in ((0, nc.vector), (1, nc.gpsimd)):
                        hs = hbv2[:, s_, :].rearrange("p (a c) -> p a c", a=1)
                        pq = bp.tile([BL, H * C], bf16, name=f"pq{s_}", tag=f"pq{s_}")
                        pq3 = pq[:].rearrange("p (k c) -> p k c", c=C)
                        eng.tensor_tensor(
                            out=pq3, in0=qs3,
                            in1=hs.to_broadcast([BL, H, C]), op=Alu.mult,
                        )
                        q1 = bp.tile([BL, H * 32], bf16, name=f"q1{s_}", tag=f"q1{s_}")
                        q1v = q1[:].rearrange("p (k c) -> p k c", c=32)
                        eng.tensor_add(q1v, pq3[:, :, 0:32], pq3[:, :, 32:64])
                        q2 = bp.tile([BL, H * 16], bf16, name=f"q2{s_}", tag=f"q2{s_}")
                        q2v = q2[:].rearrange("p (k c) -> p k c", c=16)
                        eng.tensor_add(q2v, q1v[:, :, 0:16], q1v[:, :, 16:32])
                        q3 = bp.tile([BL, H * 8], bf16, name=f"q3{s_}", tag=f"q3{s_}")
                        q3v = q3[:].rearrange("p (k c) -> p k c", c=8)
                        eng.tensor_add(q3v, q2v[:, :, 0:8], q2v[:, :, 8:16])
                        nc.vector.tensor_reduce(
                            out=bqv[:, s_, :].rearrange("p (a k) -> p a k", a=1),
                            in_=q3[:].rearrange("p (a k c) -> p a k c", a=1, c=8),
                            axis=Ax.X, op=Alu.add,
                        )

                    def matvec(roff, a3t, tag):
                        rs3 = qv[:, roff : roff + H * JP].rearrange(
                            "p (k j) -> p k j", j=JP
                        )
                        a3v3 = a3t[:].rearrange("p (s j) -> p s j", s=2)
                        cr = sp.tile([BL, 2 * H], f32, name="cr", tag=f"cr{tag}")
                        crv = cr[:].rearrange("p (s k) -> p s k", s=2)
                        for s_, eng in ((0, nc.vector), (1, nc.gpsimd)):
                            a3s2 = a3v3[:, s_, :].rearrange("p (a j) -> p a j", a=1)
                            pr = bp.tile(
                                [BL, H * JP], bf16, name=f"pr{s_}", tag=f"pr{tag}{s_}"
                            )
                            pr3 = pr[:].rearrange("p (k j) -> p k j", j=JP)
                            eng.tensor_tensor(
                                out=pr3, in0=rs3,
                                in1=a3s2.to_broadcast([BL, H, JP]), op=Alu.mult,
                            )
                            r1 = bp.tile(
                                [BL, H * 14], bf16, name=f"r1{s_}", tag=f"r1{tag}{s_}"
                            )
                            r1v = r1[:].rearrange("p (k j) -> p k j", j=14)
                            eng.tensor_add(r1v, pr3[:, :, 0:14], pr3[:, :, 14:28])
                            r2 = bp.tile(
                                [BL, H * 7], bf16, name=f"r2{s_}", tag=f"r2{tag}{s_}"
                            )
                            r2v = r2[:].rearrange("p (k j) -> p k j", j=7)
                            eng.tensor_add(r2v, r1v[:, :, 0:7], r1v[:, :, 7:14])
                            nc.vector.tensor_reduce(
                                out=crv[:, s_, :].rearrange("p (a k) -> p a k", a=1),
                                in_=r2[:].rearrange("p (a k j) -> p a k j", a=1, j=7),
                                axis=Ax.X, op=Alu.add,
                            )
                        return cr

                    # single sweep: a2 = probeE + Rsum @ a3_{m-1}
                    cr1 = matvec(o2, a3bf_prev, "1")
                    a2a = sp.tile([BL, 2 * H], f32, name="a2a", tag="a2a")
                    nc.vector.tensor_add(a2a[:], bq[:], cr1[:])
                    a2b_ = a2a[:].rearrange("p (s k a) -> p s k a", s=2, a=1)
                    wv = bp.tile([BL, 2 * H * 4], f32, name="wv1", tag="wv1")
                    wv4 = wv[:].rearrange("p (s k i) -> p s k i", s=2, i=4)
                    nc.vector.tensor_tensor(
                        out=wv4,
                        in0=ea4,
                        in1=a2b_.to_broadcast([BL, 2, H, 4]),
                        op=Alu.add,
                    )
                    e = bp.tile([BL, 2 * H * 4], f32, name="e1", tag="e1")
                    nc.scalar.activation(e[:], wv[:], Act.Exp)
                    e4 = e[:].rearrange("p (s k i) -> p s k i", s=2, i=4)
                    s3 = sp.tile([BL, 2 * H], f32, name="s31", tag="s31")
                    nc.gpsimd.tensor_add(
                        s3[:].rearrange("p (s k) -> p k s", k=H),
                        e4[:, 0, :, 2:4],
                        e4[:, 1, :, 2:4],
                    )
                    a3bf = sp.tile([BL, 2 * JP], bf16, name="a3b", tag="a3b1")
                    nc.scalar.memzero(a3bf[:])
                    a3bfv = a3bf[:].rearrange("p (s j) -> p s j", s=2)
                    nc.scalar.activation(a3bfv[:, :, 0:H], s3[:], Act.Ln)

                    # --- V-side: h_{sb} = h_{sb-1} + sum_j V_j a3_j ---
                    # (state 0 on DVE, state 1 mult+folds on Pool; skipped for
                    # the final sub-block where h is dead)
                    if sb < NSB - 1:
                        a3v2 = a3bf[:].rearrange("p (s j) -> p s j", s=2)
                        hprev = hbufs[(sb + 1) % 2]
                        hdst = hbufs[sb % 2]
                        vsc = qv[:, o1 : o1 + C * JP].rearrange(
                            "p (c j) -> p c j", j=JP
                        )
                        dz = sp.tile([BL, 2 * C], f32, name="dz", tag="dz")
                        dzv = dz[:].rearrange("p (s c) -> p s c", s=2)
                        for s_, eng in ((0, nc.vector), (1, nc.gpsimd)):
                            a3s = a3v2[:, s_, :].rearrange("p (a j) -> p a j", a=1)
                            pv = bp.tile([BL, C * JP], bf16, name=f"pv{s_}", tag=f"pv{s_}")
                            pvv = pv[:].rearrange("p (c j) -> p c j", j=JP)
                            eng.tensor_tensor(
                                out=pvv, in0=vsc,
                                in1=a3s.to_broadcast([BL, C, JP]), op=Alu.mult,
                            )
                            g1 = bp.tile([BL, C * 14], bf16, name=f"g1{s_}", tag=f"g1{s_}")
                            g1v = g1[:].rearrange("p (c j) -> p c j", j=14)
                            eng.tensor_add(g1v, pvv[:, :, 0:14], pvv[:, :, 14:28])
                            g2 = bp.tile([BL, C * 7], bf16, name=f"g2{s_}", tag=f"g2{s_}")
                            g2v = g2[:].rearrange("p (c j) -> p c j", j=7)
                            eng.tensor_add(g2v, g1v[:, :, 0:7], g1v[:, :, 7:14])
                            nc.vector.tensor_reduce(
                                out=dzv[:, s_, :].rearrange("p (a c) -> p a c", a=1),
                                in_=g2[:].rearrange("p (a c j) -> p a c j", a=1, j=7),
                                axis=Ax.X, op=Alu.add,
                            )
                        if m == NSBW - 1:
                            hsum = sp.tile([BL, 2 * C], f32, name="hs", tag="hs")
                            nc.vector.tensor_add(hsum[:], hprev[:], dz[:])
                            pv1 = pwt[:, w, :].rearrange("p (a c) -> p a c", a=1)
                            nc.vector.tensor_tensor(
                                out=hdst[:].rearrange("p (s c) -> p s c", s=2),
                                in0=hsum[:].rearrange("p (s c) -> p s c", s=2),
                                in1=pv1.to_broadcast([BL, 2, C]),
                                op=Alu.mult,
                            )
                        else:
                            nc.gpsimd.tensor_add(hdst[:], hprev[:], dz[:])
                    a3bf_prev = a3bf

                    # --- outputs from final e ---
                    e4 = e[:].rearrange("p (s k i) -> p s k i", s=2, i=4)
                    ssb = sp.tile([BL, H * 3], f32, name="ssb", tag="ssb")
                    ssb3 = ssb[:].rearrange("p (k i) -> p k i", i=3)
                    nc.gpsimd.tensor_add(
                        ssb3[:, :, 0:2], e4[:, 0, :, 0:2], e4[:, 1, :, 0:2]
                    )
                    nc.gpsimd.tensor_add(
                        ssb3[:, :, 2:3], ssb3[:, :, 0:1], ssb3[:, :, 1:2]
                    )
                    ll = sp.tile([BL, H * 3], f32, name="ll", tag="ll")
                    nc.scalar.activation(ll[:], ssb[:], Act.Ln)
                    ll3 = ll[:].rearrange("p (k i) -> p k i", i=3)
                    ob = sp.tile([BL, H * 2], f32, name="ob", tag="ob")
                    nc.vector.tensor_tensor(
                        out=ob[:].rearrange("p (k i) -> p k i", i=2),
                        in0=ll3[:, :, 0:2],
                        in1=ll3[:, :, 2:3].to_broadcast([BL, H, 2]),
                        op=Alu.subtract,
                    )
                    nc.sync.dma_start(out[:, sb * 2 * H : (sb + 1) * 2 * H], ob[:])
    nc.compile()
    names = dict(
        qvr=qvr.tensor.name, eaxs=eaxs.tensor.name, lainit=lainit.tensor.name,
        pws=pws.tensor.name, out=out.tensor.name,
    )
    return nc, names


def _get_program():
    with _lock:
        if "nc" not in _cache:
            _cache["nc"], _cache["names"] = _build_program()
    return _cache["nc"], _cache["names"]


def out_tensor_name(nc):
    return _cache["names"]["out"]


def _log_softmax(x, axis):
    x = np.asarray(x, np.float64)
    m = x.max(axis=axis, keepdims=True)
    e = np.exp(x - m)
    return x - m - np.log(e.sum(axis=axis, keepdims=True))


def _host_prep(corr, kc, A, trans_logits, obs_logits, init_logits):
    import ml_dtypes

    bf = ml_dtypes.bfloat16
    A64 = np.asarray(A, np.float64)
    log_obs = _log_softmax(obs_logits, 2)
    log_t = _log_softmax(trans_logits, 1)
    log_i = _log_softmax(init_logits, 1)
    AW = (A64 @ log_obs.reshape(C, 4)).astype(np.float32)
    AT = (A64 @ log_t.reshape(C, 4)).astype(np.float32)
    kc_ = np.asarray(kc)
    y = np.asarray(corr)

    cc = np.asarray(A, np.float32)[kc_]
    ccw = cc.reshape(B, NW, G, C)
    cpi = np.cumprod(1.0 - ccw, axis=2, dtype=np.float32)
    cpe = np.concatenate(
        [np.ones((B, NW, 1, C), np.float32), cpi[:, :, :-1]], axis=2
    )
    Q = (ccw * cpe).reshape(B, NSB, H, C)      # per sub-block
    V = (ccw / cpi).reshape(B, NSB, H, C)
    P = cpi[:, :, -1]                           # [B,NW,C]

    # probe matrices: window-opening sub-blocks get Q * P_{w-1}
    Qeff = Q.copy()
    for w in range(1, NW):
        Qeff[:, w * NSBW] = Q[:, w * NSBW] * P[:, w - 1][:, None, :]

    # R_tri (in-sub-block strictly lower) and R_cross (vs previous sub-block)
    Rtri = np.matmul(
        Q.reshape(-1, H, C), V.reshape(-1, H, C).transpose(0, 2, 1)
    ).reshape(B, NSB, H, H)
    Rtri *= np.tril(np.ones((H, H), np.float32), -1)
    Rcross = np.zeros((B, NSB, H, H), np.float32)
    Qe = Qeff.reshape(B, NSB, H, C)
    for sbi in range(1, NSB):
        Rcross[:, sbi] = np.matmul(
            Qe[:, sbi], V[:, sbi - 1].transpose(0, 2, 1)
        )
    Rsum = Rcross + Rtri

    eax = np.empty((B, T, 2, 4), np.float32)
    for tp in range(2):
        eax[:, :, tp, 0] = AW[kc_, tp * 2 + 0]
        eax[:, :, tp, 1] = AW[kc_, tp * 2 + 1]
        awy = AW[kc_, tp * 2 + y]
        for s in range(2):
            eax[:, :, tp, 2 + s] = awy + AT[kc_, s * 2 + tp]
    eaxs = (
        eax.reshape(B, NSB, H, 2, 4).transpose(0, 1, 3, 2, 4).reshape(B, NSB, -1)
    )

    qvr = np.zeros((B, NSB, SBW), bf)
    qvr[:, :, 0 : H * C] = Qeff.reshape(B, NSB, H * C).astype(bf)
    Vp = np.zeros((B, NSB, C, JP), np.float32)
    Vp[:, :, :, :H] = V.transpose(0, 1, 3, 2)
    o1 = H * C
    qvr[:, :, o1 : o1 + C * JP] = Vp.reshape(B, NSB, -1).astype(bf)
    Rp = np.zeros((B, NSB, H, JP), np.float32)
    Rp[:, :, :, :H] = Rsum
    o2 = o1 + C * JP
    qvr[:, :, o2 : o2 + H * JP] = Rp.reshape(B, NSB, -1).astype(bf)

    lainit = np.zeros((BL, 2 * C), np.float32)
    li = log_i.astype(np.float32)
    lainit[:, 0:C] = li[:, 0][None, :]
    lainit[:, C : 2 * C] = li[:, 1][None, :]
    return qvr, eaxs, P.astype(np.float32), lainit


def prepare_in_maps(inputs):
    nc, names = _get_program()
    qvr, eaxs, P, lainit = _host_prep(**inputs)
    in_maps = []
    for c in range(N_CORES):
        sl = slice(c * BL, (c + 1) * BL)
        in_maps.append({
            names["qvr"]: qvr[sl],
            names["eaxs"]: eaxs[sl],
            names["pws"]: P[sl],
            names["lainit"]: lainit,
        })
    return nc, in_maps


def kernel(corr, kc, A, trans_logits, obs_logits, init_logits):
    from concourse.bass_utils import run_bass_kernel_spmd

    nc, in_maps = prepare_in_maps(dict(
        corr=corr, kc=kc, A=A, trans_logits=trans_logits,
        obs_logits=obs_logits, init_logits=init_logits))
    names = _cache["names"]
    res = run_bass_kernel_spmd(nc, in_maps, core_ids=list(range(N_CORES)))
    outs = [res.results[c][names["out"]].reshape(BL, T, O) for c in range(N_CORES)]
    return np.concatenate(outs, axis=0)
